# revision 27
# baseline (speedup 1.0000x reference)
"""Trainium2 Bass kernel for nn_ChemistryAwareDecoder.

Reference computation (per edge e = (s, d)):
    sp = z[s] * z[d]                       # [128]
    cp = chem[s] * chem[d]                 # [768]
    score_s = relu(sp @ sw1 + sb1) @ sw2 + sb2
    score_c = relu(cp @ cw1 + cb1) @ cw2 + cb2
    score_m = relu(concat(sp, cp) @ mw1 + mb1) @ mw2 + mb2
    t = w0*score_s + w1*score_c + w2*score_m
    bv = mask[s] * mask[d]
    out = bv > 0.5 ? t : score_s

Strategy: data-parallel over edges across 8 NeuronCores, bf16 compute.
Each core holds a replicated padded node table [N, 1024] = [z | chem | mask |
0-pad] in DRAM. Edges are sorted by src on the host so each core's src values
fit a 32768-row window (int16 indices), and within a core edges are bucketed
by dst into 4 windows of N/4 rows (int16 again). Per 512-edge block:
  - 2 transposing dma_gathers (src rows, dst rows) -> [128 feat-part, 8, 512]
    SBUF tiles, i.e. the gathered rows arrive already transposed
  - one DVE elementwise product = transposed pair products (mask product
    lands on partition 0 of chunk 7 -> bv row for free)
  - matmuls for the 3 MLPs (first layer contracts feat chunks 0..6),
    second layer includes a ones-row that carries the score biases
  - blend on [1, 512] score rows, DMA out; host unpermutes to edge order
"""

import os
import numpy as np

N_NODES = 100000
E_TOTAL = 200000
SD = 128
CD = 768
F = SD + CD            # 896 real features
ELEM = 1024            # padded table row (bf16 -> 2048B, %256==0)
NCORES = 8
BLK = 512              # edges per block
NBUCK = 4
SRCWIN = 32768

LAST_EXEC_NS = None


def _build(n_nodes, bucket_blocks, srcwin):
    import concourse.bass as bass  # noqa: F401
    import concourse.tile as tile
    from concourse import bacc, mybir
    from concourse.tile_rust import add_dep_helper

    F32 = mybir.dt.float32
    I16 = mybir.dt.int16
    DT = mybir.dt.bfloat16
    AF = mybir.ActivationFunctionType
    OP = mybir.AluOpType

    dstwin = -(-n_nodes // NBUCK)
    nblk = sum(bucket_blocks)
    bucket_of = [g for g in range(NBUCK) for _ in range(bucket_blocks[g])]

    nc = bacc.Bacc(num_swdge_queues=2)

    table_d = nc.declare_dram_parameter("table", [n_nodes, ELEM], DT, isOutput=False)
    stable_d = nc.declare_dram_parameter("stable", [srcwin, ELEM], DT, isOutput=False)
    eidx_d = nc.declare_dram_parameter("eidx", [128, nblk * 64], I16, isOutput=False)
    sw1_d = nc.declare_dram_parameter("sw1", [128, 64], DT, isOutput=False)
    cw1a_d = nc.declare_dram_parameter("cw1a", [128, 6 * 128], DT, isOutput=False)
    cw1b_d = nc.declare_dram_parameter("cw1b", [128, 6 * 64], DT, isOutput=False)
    mw1p_d = nc.declare_dram_parameter("mw1p", [128, 7 * 128], DT, isOutput=False)
    b1_d = nc.declare_dram_parameter("b1pack", [384], F32, isOutput=False)
    w2_d = nc.declare_dram_parameter("w2pack", [450], DT, isOutput=False)
    out_d = nc.declare_dram_parameter("out", [nblk, BLK], F32, isOutput=True)

    with tile.TileContext(nc) as tc:
        with (
            tc.tile_pool(name="const", bufs=1) as cpool,
            tc.tile_pool(name="gather", bufs=3) as gpool,
            tc.tile_pool(name="prod", bufs=3) as ppool,
            tc.tile_pool(name="hid", bufs=2) as hpool,
            tc.tile_pool(name="blend", bufs=2) as bpool,
            tc.tile_pool(name="ph", bufs=2, space="PSUM") as phpool,
            tc.tile_pool(name="ps", bufs=2, space="PSUM") as pspool,
        ):
            # ---- constants, loaded once ----
            eidx_t = cpool.tile([128, nblk * 64], I16, tag="eidx")
            nc.sync.dma_start(out=eidx_t[:], in_=eidx_d[:])

            sw1_t = cpool.tile([128, 64], DT, tag="sw1")
            cw1a_t = cpool.tile([128, 6 * 128], DT, tag="cw1a")
            cw1b_t = cpool.tile([128, 6 * 64], DT, tag="cw1b")
            mw1_t = cpool.tile([128, 7 * 128], DT, tag="mw1")
            nc.sync.dma_start(out=sw1_t[:], in_=sw1_d[:])
            nc.sync.dma_start(out=cw1a_t[:], in_=cw1a_d[:])
            nc.sync.dma_start(out=cw1b_t[:], in_=cw1b_d[:])
            nc.sync.dma_start(out=mw1_t[:], in_=mw1p_d[:])

            sb1_t = cpool.tile([64, 1], F32, tag="sb1")
            cb1a_t = cpool.tile([128, 1], F32, tag="cb1a")
            cb1b_t = cpool.tile([64, 1], F32, tag="cb1b")
            mb1_t = cpool.tile([128, 1], F32, tag="mb1")
            nc.sync.dma_start(out=sb1_t[:], in_=b1_d[0:64])
            nc.sync.dma_start(out=cb1a_t[:], in_=b1_d[64:192])
            nc.sync.dma_start(out=cb1b_t[:], in_=b1_d[192:256])
            nc.sync.dma_start(out=mb1_t[:], in_=b1_d[256:384])

            # w2pack layout: s2 [65] | t2st [65] | t2cha [128] | t2chb [64] | t2cb [128]
            s2_t = cpool.tile([65, 1], DT, tag="s2")
            t2st_t = cpool.tile([65, 1], DT, tag="t2st")
            t2cha_t = cpool.tile([128, 1], DT, tag="t2cha")
            t2chb_t = cpool.tile([64, 1], DT, tag="t2chb")
            t2cb_t = cpool.tile([128, 1], DT, tag="t2cb")
            nc.sync.dma_start(out=s2_t[:], in_=w2_d[0:65])
            nc.sync.dma_start(out=t2st_t[:], in_=w2_d[65:130])
            nc.sync.dma_start(out=t2cha_t[:], in_=w2_d[130:258])
            nc.sync.dma_start(out=t2chb_t[:], in_=w2_d[258:322])
            nc.sync.dma_start(out=t2cb_t[:], in_=w2_d[322:450])

            # persistent double-buffered structural-hidden tiles; row 64 is a
            # constant ones-row (carries the layer-2 biases), written once.
            hst_bufs = [cpool.tile([65, BLK], DT, name=f"hst{i}", tag=f"hst{i}")
                        for i in range(2)]
            for t in hst_bufs:
                nc.gpsimd.memset(t[64:65, :], 1.0)

            # blend is deferred one block so the next block's product TT
            # precedes it in the DVE queue (keeps PE fed).
            def emit_blend(st):
                # out = where(bv != 0, t, s): copy s then overwrite where bv
                pscore, prodT, bb = st
                o_t = bpool.tile([1, BLK], F32, tag="o")
                nc.vector.tensor_copy(out=o_t[:], in_=pscore[32:33, :])
                nc.vector.copy_predicated(out=o_t[:],
                                          mask=prodT[0:1, 7 * BLK:8 * BLK]
                                          .bitcast(mybir.dt.int16),
                                          data=pscore[0:1, :])
                nc.sync.dma_start(out=out_d[bb:bb + 1, :], in_=o_t[:])

            pending = None

            # ---- per-block pipeline ----
            for b in range(nblk):
                g = bucket_of[b]
                q_src, q_dst = 0, 1
                # transposing gathers: out[a, c, i] = table[idx_i, c*128 + a]
                srcT = gpool.tile([128, 8 * BLK], DT, tag="srcT")
                dstT = gpool.tile([128, 8 * BLK], DT, tag="dstT")
                nc.gpsimd.dma_gather(
                    out_ap=srcT[:].rearrange("p (c e) -> p c e", e=BLK),
                    in_ap=stable_d[:],
                    idxs_ap=eidx_t[:, b * 64:b * 64 + 32],
                    num_idxs=BLK, num_idxs_reg=BLK,
                    elem_size=ELEM, transpose=True,
                    queue_num=q_src,
                )
                nc.gpsimd.dma_gather(
                    out_ap=dstT[:].rearrange("p (c e) -> p c e", e=BLK),
                    in_ap=table_d[g * dstwin:(g + 1) * dstwin, :],
                    idxs_ap=eidx_t[:, b * 64 + 32:b * 64 + 64],
                    num_idxs=BLK, num_idxs_reg=BLK,
                    elem_size=ELEM, transpose=True,
                    queue_num=q_dst,
                )

                # pair products, already in [feat, edge] layout; chunk 7 row 0
                # is mask_src*mask_dst = bv.
                prodT = ppool.tile([128, 8 * BLK], DT, tag="prodT")
                nc.vector.tensor_tensor(
                    out=prodT[:], in0=srcT[:], in1=dstT[:], op=OP.mult)

                # first layers (contract feat chunks: 0 structural, 1..6 chem)
                # st and chb share one PSUM bank (rows 0:64 / 64:128); st's
                # bank-clearing start=True must precede chb's accumulation.
                pstb = phpool.tile([128, BLK], F32, tag="pstb")
                i_st = nc.tensor.matmul(pstb[0:64, :], lhsT=sw1_t[:],
                                        rhs=prodT[:, 0:BLK],
                                        start=True, stop=True)
                p_cha = phpool.tile([128, BLK], F32, tag="pcha")
                for k in range(6):
                    nc.tensor.matmul(
                        p_cha[:], lhsT=cw1a_t[:, k * 128:(k + 1) * 128],
                        rhs=prodT[:, (k + 1) * BLK:(k + 2) * BLK],
                        start=(k == 0), stop=(k == 5))
                for k in range(6):
                    i_mm = nc.tensor.matmul(
                        pstb[64:128, :], lhsT=cw1b_t[:, k * 64:(k + 1) * 64],
                        rhs=prodT[:, (k + 1) * BLK:(k + 2) * BLK],
                        start=(k == 0), stop=(k == 5))
                    if k == 0:
                        add_dep_helper(i_mm.ins, i_st.ins, sync=False,
                                       reason="st bank-clear before chb accum")
                p_cb = phpool.tile([128, BLK], F32, tag="pcb")
                for k in range(7):
                    nc.tensor.matmul(
                        p_cb[:], lhsT=mw1_t[:, k * 128:(k + 1) * 128],
                        rhs=prodT[:, k * BLK:(k + 1) * BLK],
                        start=(k == 0), stop=(k == 6))

                # hidden activations (relu + bias)
                hid_st = hst_bufs[b % 2]
                nc.scalar.activation(out=hid_st[0:64, :], in_=pstb[0:64, :],
                                     func=AF.Relu, bias=sb1_t[:])
                hid_cha = hpool.tile([128, BLK], DT, tag="hcha")
                nc.scalar.activation(out=hid_cha[:], in_=p_cha[:],
                                     func=AF.Relu, bias=cb1a_t[:])
                hid_chb = hpool.tile([64, BLK], DT, tag="hchb")
                nc.scalar.activation(out=hid_chb[:], in_=pstb[64:128, :],
                                     func=AF.Relu, bias=cb1b_t[:])
                hid_cb = hpool.tile([128, BLK], DT, tag="hcb")
                nc.scalar.activation(out=hid_cb[:], in_=p_cb[:],
                                     func=AF.Relu, bias=mb1_t[:])

                # second layer: t at row 0, s at row 32 of one shared bank;
                # s's bank-clearing start=True precedes t's accumulation group.
                pscore = pspool.tile([128, BLK], F32, tag="pscore")
                i_psc = nc.tensor.matmul(pscore[32:33, :], lhsT=s2_t[:],
                                         rhs=hid_st[:], start=True, stop=True)
                i_pt1 = nc.tensor.matmul(pscore[0:1, :], lhsT=t2st_t[:],
                                         rhs=hid_st[:], start=True, stop=False)
                add_dep_helper(i_pt1.ins, i_psc.ins, sync=False,
                               reason="s bank-clear before t accum")
                nc.tensor.matmul(pscore[0:1, :], lhsT=t2cha_t[:], rhs=hid_cha[:],
                                 start=False, stop=False)
                nc.tensor.matmul(pscore[0:1, :], lhsT=t2chb_t[:], rhs=hid_chb[:],
                                 start=False, stop=False)
                nc.tensor.matmul(pscore[0:1, :], lhsT=t2cb_t[:], rhs=hid_cb[:],
                                 start=False, stop=True)

                # blend of the PREVIOUS block: out = where(bv, t, s)
                if pending is not None:
                    emit_blend(pending)
                pending = (pscore, prodT, b)

            emit_blend(pending)

    nc.finalize()
    return nc


def _host_prep(z, chemistry, edge, smiles_mask,
               sw1, sb1, sw2, sb2, cw1, cb1, cw2, cb2, mw1, mb1, mw2, mb2,
               path_weights, n_nodes=N_NODES, ncores=NCORES):
    """Sort/bucket edges, build the padded bf16 table + per-core shards."""
    import ml_dtypes
    wdt = ml_dtypes.bfloat16

    z = np.asarray(z, np.float32)
    chemistry = np.asarray(chemistry, np.float32)
    mask = np.asarray(smiles_mask, np.float32).reshape(-1)
    table = np.zeros((n_nodes, ELEM), np.float32)
    table[:, :SD] = z
    table[:, SD:F] = chemistry
    table[:, F] = mask
    table = table.astype(wdt)

    srcwin = min(SRCWIN, n_nodes)
    dstwin = -(-n_nodes // NBUCK)
    assert dstwin <= 32767

    pw = np.asarray(path_weights, np.float64)
    e = np.exp(pw - pw.max())
    w = e / e.sum()
    w0, w1, w2 = [float(x) for x in w]

    sw1 = np.asarray(sw1, np.float32)
    cw1 = np.asarray(cw1, np.float32)
    mw1 = np.asarray(mw1, np.float32)
    cw1a = cw1[:, :128].reshape(6, 128, 128).transpose(1, 0, 2).reshape(128, 6 * 128)
    cw1b = cw1[:, 128:].reshape(6, 128, 64).transpose(1, 0, 2).reshape(128, 6 * 64)
    mw1p = mw1.reshape(7, 128, 128).transpose(1, 0, 2).reshape(128, 7 * 128)
    b1pack = np.concatenate([
        np.asarray(sb1, np.float32),
        np.asarray(cb1, np.float32)[:128],
        np.asarray(cb1, np.float32)[128:],
        np.asarray(mb1, np.float32)]).astype(np.float32)

    sw2v = np.asarray(sw2, np.float64).reshape(-1)
    cw2v = np.asarray(cw2, np.float64).reshape(-1)
    mw2v = np.asarray(mw2, np.float64).reshape(-1)
    sb2v = float(np.asarray(sb2, np.float64).reshape(())[()])
    cb2v = float(np.asarray(cb2, np.float64).reshape(())[()])
    mb2v = float(np.asarray(mb2, np.float64).reshape(())[()])
    tb = w0 * sb2v + w1 * cb2v + w2 * mb2v
    w2pack = np.concatenate([
        np.concatenate([sw2v, [sb2v]]),
        np.concatenate([w0 * sw2v, [tb]]),
        w1 * cw2v[:128], w1 * cw2v[128:], w2 * mw2v]).astype(np.float32)
    assert w2pack.shape == (450,)

    edge = np.asarray(edge)
    E = edge.shape[0]
    src_all = edge[:, 0].astype(np.int64)
    dst_all = edge[:, 1].astype(np.int64)
    order = np.argsort(src_all, kind='stable')
    epc = E // ncores

    cores = []
    counts_all = np.zeros((ncores, NBUCK), np.int64)
    for c in range(ncores):
        ids = order[c * epc:(c + 1) * epc]
        s = src_all[ids]
        d = dst_all[ids]
        w0c = max(0, min(int(s.min()), n_nodes - srcwin))
        assert int(s.max()) - w0c < srcwin, "src window overflow"
        g = d // dstwin
        bord = np.argsort(g, kind='stable')
        ids, s, d, g = ids[bord], s[bord], d[bord], g[bord]
        counts_all[c] = np.bincount(g, minlength=NBUCK)
        cores.append((ids, s - w0c, d - g * dstwin, g, w0c))

    bucket_blocks = tuple(int(-(-int(counts_all[:, gg].max()) // BLK))
                          for gg in range(NBUCK))
    bucket_blocks = tuple(max(1, bb) for bb in bucket_blocks)
    nblk = sum(bucket_blocks)

    shards = []
    for c in range(ncores):
        ids, s_rel, d_rel, g, w0c = cores[c]
        src16 = np.zeros(nblk * BLK, np.int16)
        dst16 = np.zeros(nblk * BLK, np.int16)
        perm = np.full(nblk * BLK, -1, np.int64)
        base_blk = 0
        pos = 0
        for gg in range(NBUCK):
            n_g = int(counts_all[c, gg])
            sl = slice(base_blk * BLK, base_blk * BLK + n_g)
            src16[sl] = s_rel[pos:pos + n_g].astype(np.int16)
            dst16[sl] = d_rel[pos:pos + n_g].astype(np.int16)
            perm[sl] = ids[pos:pos + n_g]
            pos += n_g
            base_blk += bucket_blocks[gg]
        # per-block idx wrap: flat pos k -> [k%16, k//16], replicated x8
        ar = np.arange(BLK)
        eidx = np.zeros((16, nblk * 64), np.int16)
        for b in range(nblk):
            sblk = src16[b * BLK:(b + 1) * BLK]
            dblk = dst16[b * BLK:(b + 1) * BLK]
            eidx[ar % 16, b * 64 + ar // 16] = sblk
            eidx[ar % 16, b * 64 + 32 + ar // 16] = dblk
        eidx = np.tile(eidx, (8, 1))
        stable = np.ascontiguousarray(table[w0c:w0c + srcwin])
        shards.append((eidx, stable, perm))

    shared = dict(table=table, sw1=sw1.astype(wdt),
                  cw1a=np.ascontiguousarray(cw1a).astype(wdt),
                  cw1b=np.ascontiguousarray(cw1b).astype(wdt),
                  mw1p=np.ascontiguousarray(mw1p).astype(wdt),
                  b1pack=b1pack, w2pack=w2pack.astype(wdt))
    return shared, shards, bucket_blocks, srcwin, E


_BUILD_CACHE = {}


def kernel(z, chemistry, edge, smiles_mask,
           sw1, sb1, sw2, sb2, cw1, cb1, cw2, cb2, mw1, mb1, mw2, mb2,
           path_weights):
    global LAST_EXEC_NS
    from concourse import bass_utils
    from concourse.bass_utils import run_bass_kernel_spmd

    trace = os.environ.get("KERNEL_TRACE", "0") == "1"
    if trace:
        # No artifact bucket in this container; keep the NTFF trace local.
        bass_utils.upload_artifacts = lambda tmpdir: tmpdir

    shared, shards, bucket_blocks, srcwin, E = _host_prep(
        z, chemistry, edge, smiles_mask, sw1, sb1, sw2, sb2,
        cw1, cb1, cw2, cb2, mw1, mb1, mw2, mb2, path_weights)

    key = (N_NODES, bucket_blocks, srcwin)
    if key not in _BUILD_CACHE:
        _BUILD_CACHE[key] = _build(N_NODES, bucket_blocks, srcwin)
    nc = _BUILD_CACHE[key]

    in_maps = []
    for c in range(NCORES):
        m = dict(shared)
        m["eidx"], m["stable"], _ = shards[c]
        in_maps.append(m)

    tmpdir = os.environ.get("KERNEL_TRACE_DIR") or None
    res = run_bass_kernel_spmd(nc, in_maps, core_ids=list(range(NCORES)),
                               trace=trace, tmpdir=tmpdir)
    if trace:
        LAST_EXEC_NS = res.exec_time_ns

    result = np.zeros(E, np.float32)
    for c in range(NCORES):
        perm = shards[c][2]
        dev = res.results[c]["out"].reshape(-1)
        valid = perm >= 0
        result[perm[valid]] = dev[valid]
    return result
